# revision 19
# baseline (speedup 1.0000x reference)
"""Trainium2 Bass kernel for a 2-layer LSTM extractor.

Reference computation (see problem):
  x: [512, 1, 512, 28] -> squeeze -> [B=512, T=512, D=28]
  layer0: LSTM(D=28 -> H=128), layer1: LSTM(128 -> 128)
  output: final hidden state of layer1, [512, 128]

Strategy:
  - Data parallel: batch 512 sharded 8 ways -> B=64 per NeuronCore.
  - Per core, both layers fused in one time loop, layer1 skewed one step
    behind layer0 so its work fills engine gaps.
  - Gate-transposed layout everywhere: states h/c stored [H=128 part, B=64
    free]; gate pre-activations computed as [4H-chunk part, B free] via
    matmuls with stationary weight chunks lhsT=[K,128] and moving rhs=h.
    No per-step transposes anywhere.
  - All PE operands in fp16 (1 cycle/row vs 4 for fp32; 10-bit mantissa
    keeps the 512-step recurrence well inside the 2e-2 gate).
  - L0 biases folded into the x-projection via a ones-row augmentation.
  - L1 biases injected into PSUM via a K=4 bias matmul (per-gate bias rows
    against a one-hot moving operand), so both layers use the merged
    sigmoid[3B]+tanh[B] activation pattern.
  - Cell states of both layers live in one [128, 2B] tile -> single tanh.
  - x is transposed on-chip (PE transpose) into [33, B*T] quarters;
    the per-step x-projection rhs is a strided column view.
"""

import os
import sys

import numpy as np

for _p in ("/opt/trn_rl_repo", os.path.expanduser("~/.axon_site/_ro/trn_rl_repo")):
    if os.path.isdir(_p) and _p not in sys.path:
        sys.path.insert(0, _p)

import concourse.bacc as bacc
import concourse.tile as tile
from concourse import masks, mybir
from concourse.bass_utils import run_bass_kernel_spmd

B_FULL, T_FULL, D, H = 512, 512, 28, 128
NCORES = 8
B = B_FULL // NCORES  # 64 per core
G4 = 4 * H  # 512
P = 128
F32 = mybir.dt.float32
F16 = mybir.dt.float16
AF = mybir.ActivationFunctionType

# weight chunk g (PyTorch gate order i,f,g,o) -> psum column block.
# Sigmoid gates (i,f,o) contiguous in blocks 0..2 so one ACT op covers them;
# tanh gate g in block 3.
COL_OF = [0, 1, 3, 2]  # i->0, f->1, g->3, o->2
KA = 33  # augmented contraction dim for the L0 x-projection (28 x + pad + bias)


def _emit(nc, tc, t_steps):
    Q = 4 if t_steps % 4 == 0 and t_steps >= 4 else 1
    TQ = t_steps // Q

    x = nc.dram_tensor("x", [B, t_steps, D], F32, kind="ExternalInput").ap()
    wih0 = nc.dram_tensor("W_ih0", [G4, D], F32, kind="ExternalInput").ap()
    whh0 = nc.dram_tensor("W_hh0", [G4, H], F32, kind="ExternalInput").ap()
    bih0 = nc.dram_tensor("b_ih0", [1, G4], F32, kind="ExternalInput").ap()
    bhh0 = nc.dram_tensor("b_hh0", [1, G4], F32, kind="ExternalInput").ap()
    wih1 = nc.dram_tensor("W_ih1", [G4, H], F32, kind="ExternalInput").ap()
    whh1 = nc.dram_tensor("W_hh1", [G4, H], F32, kind="ExternalInput").ap()
    bih1 = nc.dram_tensor("b_ih1", [4, H], F32, kind="ExternalInput").ap()
    bhh1 = nc.dram_tensor("b_hh1", [4, H], F32, kind="ExternalInput").ap()
    out = nc.dram_tensor("out", [B, H], F32, kind="ExternalOutput").ap()

    from contextlib import ExitStack

    es = ExitStack()
    with es:
        consts = es.enter_context(tc.tile_pool(name="consts", bufs=1))
        wstage = es.enter_context(tc.tile_pool(name="wstage", bufs=2))
        xload = es.enter_context(tc.tile_pool(name="xload", bufs=4))
        psx = es.enter_context(tc.tile_pool(name="psx", bufs=1, space="PSUM"))
        states = es.enter_context(tc.tile_pool(name="states", bufs=4))
        work = es.enter_context(tc.tile_pool(name="work", bufs=3))

        ident = consts.tile([P, P], F32)
        masks.make_identity(nc, ident[:])

        # ---- weight prep: transposed fp16 lhsT chunks ----
        # (psum pool scoped so its banks are free during the main loop)
        whh0T = consts.tile([P, G4], F16)
        wih1T = consts.tile([P, G4], F16)
        whh1T = consts.tile([P, G4], F16)
        with tc.tile_pool(name="pswt", bufs=2, space="PSUM") as pswt:
            for src, dst in ((whh0, whh0T), (wih1, wih1T), (whh1, whh1T)):
                for g in range(4):
                    wst = wstage.tile([P, H], F32, tag="wst")
                    nc.sync.dma_start(out=wst[:], in_=src[g * P : (g + 1) * P, :])
                    pst = pswt.tile([P, P], F32, tag="pswt")
                    nc.tensor.transpose(pst[:], wst[:], ident[:])
                    nc.scalar.copy(out=dst[:, g * P : (g + 1) * P], in_=pst[:])

        # wih0T augmented with the summed L0 bias as row 32 (compute-op start
        # partitions must be 32-aligned); rows 28..31 are zero so the
        # matching garbage rows of xT contribute nothing. K = KA = 33.
        wih0T = consts.tile([KA, G4], F16)
        nc.vector.memset(wih0T[:], 0.0)
        for g in range(4):
            wst = wstage.tile([P, D], F32, tag="wst")
            nc.sync.dma_start(out=wst[:], in_=wih0[g * P : (g + 1) * P, :])
            pst = psx.tile([D, P], F32, tag="psx")
            nc.tensor.transpose(pst[:], wst[:], ident[:])
            nc.vector.tensor_copy(wih0T[0:D, g * P : (g + 1) * P], pst[:])
        b0a = work.tile([1, G4], F32, tag="b0a")
        b0b = work.tile([1, G4], F32, tag="b0b")
        b0sum = work.tile([1, G4], F16, tag="b0sum")
        nc.sync.dma_start(out=b0a[:], in_=bih0)
        nc.sync.dma_start(out=b0b[:], in_=bhh0)
        nc.vector.tensor_add(b0sum[:], b0a[:], b0b[:])
        nc.sync.dma_start(out=wih0T[KA - 1 : KA, :], in_=b0sum[:])

        # L1 bias matmul operands: lhsT b4 [4, 128] (row r = summed bias of
        # weight chunk r), moving one-hot [4, 4B] with row r hot in column
        # block COL_OF[r].  psum += b4.T @ onehot puts bias b4[r, p] into
        # column block COL_OF[r] of partition p.
        b1a = work.tile([4, H], F32, tag="b1a")
        b1b = work.tile([4, H], F32, tag="b1b")
        nc.sync.dma_start(out=b1a[:], in_=bih1)
        nc.sync.dma_start(out=b1b[:], in_=bhh1)
        b4 = consts.tile([4, H], F16)
        nc.vector.tensor_add(b4[:], b1a[:], b1b[:])
        onehot = consts.tile([4, 4 * B], F16)
        nc.vector.memset(onehot[:], 0.0)
        ones_row = work.tile([1, B], F16, tag="ones_row")
        nc.vector.memset(ones_row[:], 1.0)
        for r in range(4):
            cb = COL_OF[r] * B
            nc.sync.dma_start(out=onehot[r : r + 1, cb : cb + B], in_=ones_row[:])

        # ---- x transpose prep, per quarter (xT stored fp16) ----
        xT = [
            consts.tile([KA, B * TQ], F16, tag=f"xT{q}", name=f"xT{q}")
            for q in range(Q)
        ]

        def emit_xprep(q, b):
            if b == 0:
                nc.vector.memset(xT[q][:], 0.0)
                nc.vector.memset(xT[q][KA - 1 : KA, :], 1.0)
            xt = xload.tile([TQ, D], F32, tag="xt")
            nc.sync.dma_start(out=xt[:], in_=x[b, q * TQ : (q + 1) * TQ, :])
            px = psx.tile([D, TQ], F32, tag="psx")
            nc.tensor.transpose(px[:], xt[:], ident[0:TQ, 0:TQ])
            dst = xT[q][0:D, b * TQ : (b + 1) * TQ]
            nc.vector.tensor_copy(dst, px[:])

        for b in range(B):
            emit_xprep(0, b)

        # 3 psum banks per layer: with only 2, iter k's early matmuls hit a
        # bank WAR against iter k-2's LAST gate reads on ACT, serializing
        # the pre-run work behind the critical chain.  Created after the
        # weight-prep pool is closed so peak usage stays within 8 banks.
        ps0p = es.enter_context(tc.tile_pool(name="ps0p", bufs=1, space="PSUM"))
        ps1p = es.enter_context(tc.tile_pool(name="ps1p", bufs=1, space="PSUM"))

        # ---- main time loop ----
        # Iter k runs: L0 step k; L1 stage A for step k-2 (sigmoids on ACT,
        # cell update on GpSimd); L1 stage B for step k-3 (tanh + out-mul,
        # queued at the head of the iter so it never blocks L0's chain).
        # Cell states are packed as gc = [tanh-gate | c] so the cell update
        # is one paired multiply [i|f]*[g|c] plus one pairwise add.
        h0t = states.tile([P, B], F16, tag="h0t", name="h0init")
        h1t = states.tile([P, B], F16, tag="h1t", name="h1init")
        gc0 = states.tile([P, 2 * B], F32, tag="gc0", name="gc0init")
        gc1 = states.tile([P, 2 * B], F32, tag="gc1", name="gc1init")
        for t_ in (h0t, h1t, gc0, gc1):
            nc.vector.memset(t_[:], 0.0)
        h0_init = h0t
        h0_tiles = {}  # step index -> h0 tile
        sifo1_prev = None

        # interleave next-quarter x prep into the loop
        prep_schedule = {}  # iter k -> list of (q, b)
        if Q > 1:
            for q in range(1, Q):
                base = (q - 1) * TQ
                for b in range(B):
                    kk = base + (b * TQ) // B
                    prep_schedule.setdefault(kk, []).append((q, b))

        for k in range(t_steps + 3):
            do0 = k < t_steps  # L0 step k
            doA = 2 <= k <= t_steps + 1  # L1 stage A (step k-2)
            doB = 3 <= k <= t_steps + 2  # L1 stage B (step k-3)
            h0_prev = h0_tiles.get(k - 1, h0_init)  # h0(k-1)
            h0_prev2 = h0_tiles.get(k - 2, h0_init)  # h0(k-2)
            gc0_cur, gc1_cur = gc0, gc1

            # ---------- L1 stage B: tanh(c1) + h1 = o1*tc1 ----------
            # (reads only iter-(k-1) results -> runs while PE waits for h0)
            if doB:
                tc1 = work.tile([P, B], F32, tag="tc1")
                nc.scalar.activation(tc1[:], gc1_cur[:, B : 2 * B], AF.Tanh)
                h1t = states.tile([P, B], F16, tag="h1t")
                nc.gpsimd.tensor_mul(h1t[:], sifo1_prev[:, 2 * B : 3 * B], tc1[:])

            # ---------- PE: psum accumulation groups ----------
            ps0 = ps0p.tile([P, 8 * B], F32, tag=f"ps0{k % 3}", name="ps0") if do0 else None
            ps1 = ps1p.tile([P, 8 * B], F32, tag=f"ps1{k % 3}", name="ps1") if doA else None

            if doA:
                # pre-runnable: bias outer product + x-projection on h0(k-2)
                nc.tensor.matmul(
                    ps1[:, 0 : 4 * B], lhsT=b4[:], rhs=onehot[:], start=True, stop=False
                )
                for g in range(4):
                    cb = COL_OF[g] * B
                    nc.tensor.matmul(
                        ps1[:, cb : cb + B],
                        lhsT=wih1T[:, g * P : (g + 1) * P],
                        rhs=h0_prev2[:],
                        start=False,
                        stop=False,
                    )
            if do0:
                # pre-runnable: L0 x projection (+bias via ones row)
                q, tl = k // TQ, k % TQ
                rhs_x = xT[q][:].rearrange("p (b t) -> p t b", t=TQ)[:, tl, :]
                for g in range(4):
                    cb = COL_OF[g] * B
                    nc.tensor.matmul(
                        ps0[:, cb : cb + B],
                        lhsT=wih0T[:, g * P : (g + 1) * P],
                        rhs=rhs_x,
                        start=(g == 0),
                        stop=False,
                    )
                # critical path: L0 recurrent projection (needs h0(k-1);
                # zeros at k=0, harmless)
                for g in range(4):
                    cb = COL_OF[g] * B
                    nc.tensor.matmul(
                        ps0[:, cb : cb + B],
                        lhsT=whh0T[:, g * P : (g + 1) * P],
                        rhs=h0_prev[:],
                        start=False,
                        stop=(g == 3),
                    )
            if doA:
                # L1 recurrence on h1(k-3), produced by stage B this iter
                for g in range(4):
                    cb = COL_OF[g] * B
                    nc.tensor.matmul(
                        ps1[:, cb : cb + B],
                        lhsT=whh1T[:, g * P : (g + 1) * P],
                        rhs=h1t[:],
                        start=False,
                        stop=(g == 3),
                    )

            # ---------- L0 critical chain: ACT + DVE ----------
            if do0:
                gc0 = states.tile([P, 2 * B], F32, tag="gc0")
                sifo0 = work.tile([P, 3 * B], F32, tag="sifo0")
                # i,f first (feed the cell update); o later (only needed
                # after tanh(c))
                nc.scalar.activation(sifo0[:, 0 : 2 * B], ps0[:, 0 : 2 * B], AF.Sigmoid)
                nc.scalar.activation(gc0_cur[:, 0:B], ps0[:, 3 * B : 4 * B], AF.Tanh)
                nc.scalar.activation(
                    sifo0[:, 2 * B : 3 * B], ps0[:, 2 * B : 3 * B], AF.Sigmoid
                )
                pm0 = work.tile([P, 2 * B], F32, tag="pm0")
                nc.vector.tensor_mul(pm0[:], sifo0[:, 0 : 2 * B], gc0_cur[:])
                nc.vector.tensor_add(gc0[:, B : 2 * B], pm0[:, 0:B], pm0[:, B : 2 * B])
                tc0 = work.tile([P, B], F32, tag="tc0")
                nc.scalar.activation(tc0[:], gc0[:, B : 2 * B], AF.Tanh)
                h0t = states.tile([P, B], F16, tag="h0t")
                nc.vector.tensor_mul(h0t[:], sifo0[:, 2 * B : 3 * B], tc0[:])
                h0_tiles[k] = h0t
                h0_tiles.pop(k - 3, None)

            # ---------- L1 stage A: ACT sigmoids + GpSimd cell ----------
            if doA:
                gc1 = states.tile([P, 2 * B], F32, tag="gc1")
                sifo1 = work.tile([P, 3 * B], F32, tag="sifo1")
                nc.scalar.activation(sifo1[:], ps1[:, 0 : 3 * B], AF.Sigmoid)
                nc.scalar.activation(gc1_cur[:, 0:B], ps1[:, 3 * B : 4 * B], AF.Tanh)
                pm1 = work.tile([P, 2 * B], F32, tag="pm1")
                nc.gpsimd.tensor_mul(pm1[:], sifo1[:, 0 : 2 * B], gc1_cur[:])
                nc.gpsimd.tensor_add(
                    gc1[:, B : 2 * B], pm1[:, 0:B], pm1[:, B : 2 * B]
                )
                sifo1_prev = sifo1

            # x prep for later quarters, queued after the chain ops
            for qb in prep_schedule.get(k, ()):
                emit_xprep(*qb)

        # ---- output: h1(T-1) -> fp32 -> transpose -> [B, H] ----
        h1f = work.tile([P, B], F32, tag="h1f")
        nc.vector.tensor_copy(h1f[:], h1t[:])
        pso = ps0p.tile([B, P], F32, tag="ps0")
        nc.tensor.transpose(pso[:], h1f[:], ident[:])
        ob = work.tile([B, P], F32, tag="ob")
        nc.vector.tensor_copy(ob[:], pso[:])
        nc.sync.dma_start(out=out, in_=ob[:])


_NC_CACHE = {}


def build_nc(t_steps=T_FULL):
    if t_steps in _NC_CACHE:
        return _NC_CACHE[t_steps]
    nc = bacc.Bacc(
        "TRN2",
        target_bir_lowering=False,
        debug=False,
        enable_asserts=False,
        num_devices=NCORES,
    )
    with tile.TileContext(nc) as tc:
        _emit(nc, tc, t_steps)
    nc.compile()
    _NC_CACHE[t_steps] = nc
    return nc


def make_in_maps(inputs, t_steps=T_FULL):
    x = np.asarray(inputs["x"], dtype=np.float32).reshape(B_FULL, T_FULL, D)
    x = x[:, :t_steps, :]
    shared = {
        "W_ih0": np.ascontiguousarray(inputs["W_ih0"], dtype=np.float32),
        "W_hh0": np.ascontiguousarray(inputs["W_hh0"], dtype=np.float32),
        "b_ih0": np.asarray(inputs["b_ih0"], np.float32).reshape(1, G4),
        "b_hh0": np.asarray(inputs["b_hh0"], np.float32).reshape(1, G4),
        "W_ih1": np.ascontiguousarray(inputs["W_ih1"], dtype=np.float32),
        "W_hh1": np.ascontiguousarray(inputs["W_hh1"], dtype=np.float32),
        "b_ih1": np.asarray(inputs["b_ih1"], np.float32).reshape(4, H),
        "b_hh1": np.asarray(inputs["b_hh1"], np.float32).reshape(4, H),
    }
    in_maps = []
    for c in range(NCORES):
        m = dict(shared)
        m["x"] = np.ascontiguousarray(x[c * B : (c + 1) * B])
        in_maps.append(m)
    return in_maps


def run(inputs, t_steps=T_FULL, trace=False, **kwargs):
    nc = build_nc(t_steps)
    in_maps = make_in_maps(inputs, t_steps)
    res = run_bass_kernel_spmd(
        nc, in_maps, core_ids=list(range(NCORES)), trace=trace, **kwargs
    )
    outs = [res.results[c]["out"] for c in range(NCORES)]
    return np.concatenate(outs, axis=0).astype(np.float32), res


def kernel(**inputs):
    out, _ = run(inputs)
    return out


# revision 34
# speedup vs baseline: 1.0296x; 1.0296x over previous
"""Trainium2 Bass kernel for a 2-layer LSTM extractor.

Reference computation (see problem):
  x: [512, 1, 512, 28] -> squeeze -> [B=512, T=512, D=28]
  layer0: LSTM(D=28 -> H=128), layer1: LSTM(128 -> 128)
  output: final hidden state of layer1, [512, 128]

Strategy:
  - Data parallel: batch 512 sharded 8 ways -> B=64 per NeuronCore.
  - Per core, both layers fused in one time loop, layer1 skewed one step
    behind layer0 so its work fills engine gaps.
  - Gate-transposed layout everywhere: states h/c stored [H=128 part, B=64
    free]; gate pre-activations computed as [4H-chunk part, B free] via
    matmuls with stationary weight chunks lhsT=[K,128] and moving rhs=h.
    No per-step transposes anywhere.
  - All PE operands in fp16 (1 cycle/row vs 4 for fp32; 10-bit mantissa
    keeps the 512-step recurrence well inside the 2e-2 gate).
  - L0 biases folded into the x-projection via a ones-row augmentation.
  - L1 biases injected into PSUM via a K=4 bias matmul (per-gate bias rows
    against a one-hot moving operand), so both layers use the merged
    sigmoid[3B]+tanh[B] activation pattern.
  - Cell states of both layers live in one [128, 2B] tile -> single tanh.
  - x is transposed on-chip (PE transpose) into [33, B*T] quarters;
    the per-step x-projection rhs is a strided column view.
"""

import os
import sys

import numpy as np

for _p in ("/opt/trn_rl_repo", os.path.expanduser("~/.axon_site/_ro/trn_rl_repo")):
    if os.path.isdir(_p) and _p not in sys.path:
        sys.path.insert(0, _p)

import concourse.bacc as bacc
import concourse.tile as tile
from concourse import masks, mybir
from concourse.bass_utils import run_bass_kernel_spmd

B_FULL, T_FULL, D, H = 512, 512, 28, 128
NCORES = 8
B = B_FULL // NCORES  # 64 per core
G4 = 4 * H  # 512
P = 128
F32 = mybir.dt.float32
F16 = mybir.dt.float16
AF = mybir.ActivationFunctionType

# weight chunk g (PyTorch gate order i,f,g,o) -> psum column block.
# Sigmoid gates (i,f,o) contiguous in blocks 0..2 so one ACT op covers them;
# tanh gate g in block 3.
COL_OF = [0, 1, 3, 2]  # i->0, f->1, g->3, o->2
KA = 33  # augmented contraction dim for the L0 x-projection (28 x + pad + bias)


def _emit(nc, tc, t_steps):
    Q = 4 if t_steps % 4 == 0 and t_steps >= 4 else 1
    TQ = t_steps // Q

    x = nc.dram_tensor("x", [B, t_steps, D], F32, kind="ExternalInput").ap()
    wih0 = nc.dram_tensor("W_ih0", [G4, D], F32, kind="ExternalInput").ap()
    whh0 = nc.dram_tensor("W_hh0", [G4, H], F32, kind="ExternalInput").ap()
    bih0 = nc.dram_tensor("b_ih0", [1, G4], F32, kind="ExternalInput").ap()
    bhh0 = nc.dram_tensor("b_hh0", [1, G4], F32, kind="ExternalInput").ap()
    wih1 = nc.dram_tensor("W_ih1", [G4, H], F32, kind="ExternalInput").ap()
    whh1 = nc.dram_tensor("W_hh1", [G4, H], F32, kind="ExternalInput").ap()
    bih1 = nc.dram_tensor("b_ih1", [4, H], F32, kind="ExternalInput").ap()
    bhh1 = nc.dram_tensor("b_hh1", [4, H], F32, kind="ExternalInput").ap()
    out = nc.dram_tensor("out", [B, H], F32, kind="ExternalOutput").ap()

    from contextlib import ExitStack

    es = ExitStack()
    with es:
        consts = es.enter_context(tc.tile_pool(name="consts", bufs=1))
        wstage = es.enter_context(tc.tile_pool(name="wstage", bufs=2))
        xload = es.enter_context(tc.tile_pool(name="xload", bufs=4))
        pswt = es.enter_context(tc.tile_pool(name="pswt", bufs=2, space="PSUM"))
        psx = es.enter_context(tc.tile_pool(name="psx", bufs=2, space="PSUM"))
        ps0p = es.enter_context(tc.tile_pool(name="ps0p", bufs=2, space="PSUM"))
        ps1p = es.enter_context(tc.tile_pool(name="ps1p", bufs=2, space="PSUM"))
        states = es.enter_context(tc.tile_pool(name="states", bufs=4))
        work = es.enter_context(tc.tile_pool(name="work", bufs=3))

        ident = consts.tile([P, P], F32)
        masks.make_identity(nc, ident[:])

        # ---- weight prep: transposed fp16 lhsT chunks ----
        whh0T = consts.tile([P, G4], F16)
        wih1T = consts.tile([P, G4], F16)
        whh1T = consts.tile([P, G4], F16)
        for src, dst in ((whh0, whh0T), (wih1, wih1T), (whh1, whh1T)):
            for g in range(4):
                wst = wstage.tile([P, H], F32, tag="wst")
                nc.sync.dma_start(out=wst[:], in_=src[g * P : (g + 1) * P, :])
                pst = pswt.tile([P, P], F32, tag="pswt")
                nc.tensor.transpose(pst[:], wst[:], ident[:])
                nc.scalar.copy(out=dst[:, g * P : (g + 1) * P], in_=pst[:])

        # wih0T augmented with the summed L0 bias as row 32 (compute-op start
        # partitions must be 32-aligned); rows 28..31 are zero so the
        # matching garbage rows of xT contribute nothing. K = KA = 33.
        wih0T = consts.tile([KA, G4], F16)
        nc.vector.memset(wih0T[:], 0.0)
        for g in range(4):
            wst = wstage.tile([P, D], F32, tag="wst")
            nc.sync.dma_start(out=wst[:], in_=wih0[g * P : (g + 1) * P, :])
            pst = psx.tile([D, P], F32, tag="psx")
            nc.tensor.transpose(pst[:], wst[:], ident[:])
            nc.vector.tensor_copy(wih0T[0:D, g * P : (g + 1) * P], pst[:])
        b0a = work.tile([1, G4], F32, tag="b0a")
        b0b = work.tile([1, G4], F32, tag="b0b")
        b0sum = work.tile([1, G4], F16, tag="b0sum")
        nc.sync.dma_start(out=b0a[:], in_=bih0)
        nc.sync.dma_start(out=b0b[:], in_=bhh0)
        nc.vector.tensor_add(b0sum[:], b0a[:], b0b[:])
        nc.sync.dma_start(out=wih0T[KA - 1 : KA, :], in_=b0sum[:])

        # L1 bias matmul operands: lhsT b4 [4, 128] (row r = summed bias of
        # weight chunk r), moving one-hot [4, 4B] with row r hot in column
        # block COL_OF[r].  psum += b4.T @ onehot puts bias b4[r, p] into
        # column block COL_OF[r] of partition p.
        b1a = work.tile([4, H], F32, tag="b1a")
        b1b = work.tile([4, H], F32, tag="b1b")
        nc.sync.dma_start(out=b1a[:], in_=bih1)
        nc.sync.dma_start(out=b1b[:], in_=bhh1)
        b4 = consts.tile([4, H], F16)
        nc.vector.tensor_add(b4[:], b1a[:], b1b[:])
        onehot = consts.tile([4, 4 * B], F16)
        nc.vector.memset(onehot[:], 0.0)
        ones_row = work.tile([1, B], F16, tag="ones_row")
        nc.vector.memset(ones_row[:], 1.0)
        for r in range(4):
            cb = COL_OF[r] * B
            nc.sync.dma_start(out=onehot[r : r + 1, cb : cb + B], in_=ones_row[:])

        # ---- x transpose prep, per quarter (xT stored fp16) ----
        xT = [
            consts.tile([KA, B * TQ], F16, tag=f"xT{q}", name=f"xT{q}")
            for q in range(Q)
        ]

        def emit_xprep(q, b):
            if b == 0:
                nc.vector.memset(xT[q][:], 0.0)
                nc.vector.memset(xT[q][KA - 1 : KA, :], 1.0)
            xt = xload.tile([TQ, D], F32, tag="xt")
            nc.sync.dma_start(out=xt[:], in_=x[b, q * TQ : (q + 1) * TQ, :])
            px = psx.tile([D, TQ], F32, tag="psx")
            nc.tensor.transpose(px[:], xt[:], ident[0:TQ, 0:TQ])
            dst = xT[q][0:D, b * TQ : (b + 1) * TQ]
            nc.vector.tensor_copy(dst, px[:])

        for b in range(B):
            emit_xprep(0, b)

        # ---- main time loop; L0 at t=k, L1 at t=k-2 ----
        # The 2-step lag lets every L1 matmul (bias, h1-recurrence on
        # h1(k-3), x-projection on h0(k-2)) run before h0(k-1) even exists,
        # so only L0's recurrence sits on the critical path.  L1's
        # elementwise runs on the otherwise-idle GpSimd engine; its tanh
        # stays on ACT but is queued after L0's critical ops.
        # h01 [128, 2B] fp16: cols 0:B = h0(k), B:2B = h1(k-2).
        # c01 [128, 2B] fp32: same split.
        h01 = states.tile([P, 2 * B], F16, tag="h01")
        c01 = states.tile([P, 2 * B], F32, tag="c01")
        nc.vector.memset(h01[:], 0.0)
        nc.vector.memset(c01[:], 0.0)
        h01_prev = h01  # becomes h01(k-2) view source below

        # interleave next-quarter x prep into the loop
        prep_schedule = {}  # iter k -> list of (q, b)
        if Q > 1:
            for q in range(1, Q):
                base = (q - 1) * TQ
                for b in range(B):
                    kk = base + (b * TQ) // B
                    prep_schedule.setdefault(kk, []).append((q, b))

        for k in range(t_steps + 2):
            h01_prev2, h01_prev, c01_prev = h01_prev, h01, c01
            h0_prev = h01_prev[:, 0:B]  # h0(k-1)
            h1_prev = h01_prev[:, B : 2 * B]  # h1(k-3)
            h0_prev2 = h01_prev2[:, 0:B]  # h0(k-2)

            do0 = k < t_steps
            do1 = k >= 2

            # ---------- PE: psum accumulation groups ----------
            ps0 = ps0p.tile([P, 4 * B], F32, tag="ps0", name="ps0") if do0 else None
            ps1 = ps1p.tile([P, 4 * B], F32, tag="ps1", name="ps1") if do1 else None

            if do1:
                # all of L1's matmuls depend only on >=2-iter-old state:
                # they pre-run while L0's chain finishes the previous step
                nc.tensor.matmul(
                    ps1[:], lhsT=b4[:], rhs=onehot[:], start=True, stop=False
                )
                for g in range(4):
                    cb = COL_OF[g] * B
                    nc.tensor.matmul(
                        ps1[:, cb : cb + B],
                        lhsT=whh1T[:, g * P : (g + 1) * P],
                        rhs=h1_prev,
                        start=False,
                        stop=False,
                    )
                for g in range(4):
                    cb = COL_OF[g] * B
                    nc.tensor.matmul(
                        ps1[:, cb : cb + B],
                        lhsT=wih1T[:, g * P : (g + 1) * P],
                        rhs=h0_prev2,
                        start=False,
                        stop=(g == 3),
                    )
            if do0:
                # early L0 work: x projection (+bias via ones row)
                q, tl = k // TQ, k % TQ
                rhs_x = xT[q][:].rearrange("p (b t) -> p t b", t=TQ)[:, tl, :]
                for g in range(4):
                    cb = COL_OF[g] * B
                    nc.tensor.matmul(
                        ps0[:, cb : cb + B],
                        lhsT=wih0T[:, g * P : (g + 1) * P],
                        rhs=rhs_x,
                        start=(g == 0),
                        stop=False,
                    )
                # critical path: L0 recurrent projection (needs h0_prev;
                # zeros at k=0, harmless)
                for g in range(4):
                    cb = COL_OF[g] * B
                    nc.tensor.matmul(
                        ps0[:, cb : cb + B],
                        lhsT=whh0T[:, g * P : (g + 1) * P],
                        rhs=h0_prev,
                        start=False,
                        stop=(g == 3),
                    )

            # ---------- elementwise; new state tiles ----------
            h01 = states.tile([P, 2 * B], F16, tag="h01")
            c01 = states.tile([P, 2 * B], F32, tag="c01")

            # --- L0 critical chain: ACT (split sigmoid) + DVE ---
            if do0:
                sifo0 = work.tile([P, 3 * B], F32, tag="sifo0")
                # i,f first (feed fc/ig); o last (only needed after tanh(c))
                nc.scalar.activation(sifo0[:, 0 : 2 * B], ps0[:, 0 : 2 * B], AF.Sigmoid)
                tg0 = work.tile([P, B], F32, tag="tg0")
                nc.scalar.activation(tg0[:], ps0[:, 3 * B : 4 * B], AF.Tanh)
                nc.scalar.activation(
                    sifo0[:, 2 * B : 3 * B], ps0[:, 2 * B : 3 * B], AF.Sigmoid
                )
                fc0 = work.tile([P, B], F32, tag="fc0")
                nc.vector.tensor_mul(fc0[:], sifo0[:, B : 2 * B], c01_prev[:, 0:B])
                ig0 = work.tile([P, B], F32, tag="ig0")
                nc.vector.tensor_mul(ig0[:], sifo0[:, 0:B], tg0[:])
                nc.vector.tensor_add(c01[:, 0:B], fc0[:], ig0[:])
                tc0 = work.tile([P, B], F32, tag="tc0")
                nc.scalar.activation(tc0[:], c01[:, 0:B], AF.Tanh)
                nc.vector.tensor_mul(h01[:, 0:B], sifo0[:, 2 * B : 3 * B], tc0[:])

            # --- L1 off-path: ACT sigmoids + GpSimd elementwise ---
            if do1:
                sifo1 = work.tile([P, 3 * B], F32, tag="sifo1")
                nc.scalar.activation(sifo1[:], ps1[:, 0 : 3 * B], AF.Sigmoid)
                tg1 = work.tile([P, B], F32, tag="tg1")
                nc.scalar.activation(tg1[:], ps1[:, 3 * B : 4 * B], AF.Tanh)
                fc1 = work.tile([P, B], F32, tag="fc1")
                nc.gpsimd.tensor_mul(fc1[:], sifo1[:, B : 2 * B], c01_prev[:, B : 2 * B])
                ig1 = work.tile([P, B], F32, tag="ig1")
                nc.gpsimd.tensor_mul(ig1[:], sifo1[:, 0:B], tg1[:])
                nc.gpsimd.tensor_add(c01[:, B : 2 * B], fc1[:], ig1[:])
                tc1 = work.tile([P, B], F32, tag="tc1")
                nc.scalar.activation(tc1[:], c01[:, B : 2 * B], AF.Tanh)
                nc.gpsimd.tensor_mul(h01[:, B : 2 * B], sifo1[:, 2 * B : 3 * B], tc1[:])
            else:
                nc.vector.memset(c01[:, B : 2 * B], 0.0)
                nc.vector.memset(h01[:, B : 2 * B], 0.0)

            # x prep for later quarters, queued after the chain ops
            for qb in prep_schedule.get(k, ()):
                emit_xprep(*qb)

        # ---- output: h1 = h01[:, B:2B] -> fp32 -> transpose -> [B, H] ----
        h1f = work.tile([P, B], F32, tag="h1f")
        nc.vector.tensor_copy(h1f[:], h01[:, B : 2 * B])
        pso = ps0p.tile([B, P], F32, tag="ps0")
        nc.tensor.transpose(pso[:], h1f[:], ident[:])
        ob = work.tile([B, P], F32, tag="ob")
        nc.vector.tensor_copy(ob[:], pso[:])
        nc.sync.dma_start(out=out, in_=ob[:])


_NC_CACHE = {}


def build_nc(t_steps=T_FULL):
    if t_steps in _NC_CACHE:
        return _NC_CACHE[t_steps]
    nc = bacc.Bacc(
        "TRN2",
        target_bir_lowering=False,
        debug=False,
        enable_asserts=False,
        num_devices=NCORES,
    )
    with tile.TileContext(nc) as tc:
        _emit(nc, tc, t_steps)
    nc.compile()
    _NC_CACHE[t_steps] = nc
    return nc


def make_in_maps(inputs, t_steps=T_FULL):
    x = np.asarray(inputs["x"], dtype=np.float32).reshape(B_FULL, T_FULL, D)
    x = x[:, :t_steps, :]
    shared = {
        "W_ih0": np.ascontiguousarray(inputs["W_ih0"], dtype=np.float32),
        "W_hh0": np.ascontiguousarray(inputs["W_hh0"], dtype=np.float32),
        "b_ih0": np.asarray(inputs["b_ih0"], np.float32).reshape(1, G4),
        "b_hh0": np.asarray(inputs["b_hh0"], np.float32).reshape(1, G4),
        "W_ih1": np.ascontiguousarray(inputs["W_ih1"], dtype=np.float32),
        "W_hh1": np.ascontiguousarray(inputs["W_hh1"], dtype=np.float32),
        "b_ih1": np.asarray(inputs["b_ih1"], np.float32).reshape(4, H),
        "b_hh1": np.asarray(inputs["b_hh1"], np.float32).reshape(4, H),
    }
    in_maps = []
    for c in range(NCORES):
        m = dict(shared)
        m["x"] = np.ascontiguousarray(x[c * B : (c + 1) * B])
        in_maps.append(m)
    return in_maps


def run(inputs, t_steps=T_FULL, trace=False, **kwargs):
    nc = build_nc(t_steps)
    in_maps = make_in_maps(inputs, t_steps)
    res = run_bass_kernel_spmd(
        nc, in_maps, core_ids=list(range(NCORES)), trace=trace, **kwargs
    )
    outs = [res.results[c]["out"] for c in range(NCORES)]
    return np.concatenate(outs, axis=0).astype(np.float32), res


def kernel(**inputs):
    out, _ = run(inputs)
    return out


# revision 35
# speedup vs baseline: 1.0482x; 1.0181x over previous
"""Trainium2 Bass kernel for a 2-layer LSTM extractor.

Reference computation (see problem):
  x: [512, 1, 512, 28] -> squeeze -> [B=512, T=512, D=28]
  layer0: LSTM(D=28 -> H=128), layer1: LSTM(128 -> 128)
  output: final hidden state of layer1, [512, 128]

Strategy:
  - Data parallel: batch 512 sharded 8 ways -> B=64 per NeuronCore.
  - Per core, both layers fused in one time loop, layer1 skewed one step
    behind layer0 so its work fills engine gaps.
  - Gate-transposed layout everywhere: states h/c stored [H=128 part, B=64
    free]; gate pre-activations computed as [4H-chunk part, B free] via
    matmuls with stationary weight chunks lhsT=[K,128] and moving rhs=h.
    No per-step transposes anywhere.
  - All PE operands in fp16 (1 cycle/row vs 4 for fp32; 10-bit mantissa
    keeps the 512-step recurrence well inside the 2e-2 gate).
  - L0 biases folded into the x-projection via a ones-row augmentation.
  - L1 biases injected into PSUM via a K=4 bias matmul (per-gate bias rows
    against a one-hot moving operand), so both layers use the merged
    sigmoid[3B]+tanh[B] activation pattern.
  - Cell states of both layers live in one [128, 2B] tile -> single tanh.
  - x is transposed on-chip (PE transpose) into [33, B*T] quarters;
    the per-step x-projection rhs is a strided column view.
"""

import os
import sys

import numpy as np

for _p in ("/opt/trn_rl_repo", os.path.expanduser("~/.axon_site/_ro/trn_rl_repo")):
    if os.path.isdir(_p) and _p not in sys.path:
        sys.path.insert(0, _p)

import concourse.bacc as bacc
import concourse.tile as tile
from concourse import masks, mybir
from concourse.bass_utils import run_bass_kernel_spmd

B_FULL, T_FULL, D, H = 512, 512, 28, 128
NCORES = 8
B = B_FULL // NCORES  # 64 per core
G4 = 4 * H  # 512
P = 128
F32 = mybir.dt.float32
F16 = mybir.dt.float16
AF = mybir.ActivationFunctionType

# weight chunk g (PyTorch gate order i,f,g,o) -> psum column block.
# Sigmoid gates (i,f,o) contiguous in blocks 0..2 so one ACT op covers them;
# tanh gate g in block 3.
COL_OF = [0, 1, 3, 2]  # i->0, f->1, g->3, o->2
KA = 33  # augmented contraction dim for the L0 x-projection (28 x + pad + bias)


def _emit(nc, tc, t_steps):
    Q = 4 if t_steps % 4 == 0 and t_steps >= 4 else 1
    TQ = t_steps // Q

    x = nc.dram_tensor("x", [B, t_steps, D], F32, kind="ExternalInput").ap()
    wih0 = nc.dram_tensor("W_ih0", [G4, D], F32, kind="ExternalInput").ap()
    whh0 = nc.dram_tensor("W_hh0", [G4, H], F32, kind="ExternalInput").ap()
    bih0 = nc.dram_tensor("b_ih0", [1, G4], F32, kind="ExternalInput").ap()
    bhh0 = nc.dram_tensor("b_hh0", [1, G4], F32, kind="ExternalInput").ap()
    wih1 = nc.dram_tensor("W_ih1", [G4, H], F32, kind="ExternalInput").ap()
    whh1 = nc.dram_tensor("W_hh1", [G4, H], F32, kind="ExternalInput").ap()
    bih1 = nc.dram_tensor("b_ih1", [4, H], F32, kind="ExternalInput").ap()
    bhh1 = nc.dram_tensor("b_hh1", [4, H], F32, kind="ExternalInput").ap()
    out = nc.dram_tensor("out", [B, H], F32, kind="ExternalOutput").ap()

    from contextlib import ExitStack

    es = ExitStack()
    with es:
        consts = es.enter_context(tc.tile_pool(name="consts", bufs=1))
        wstage = es.enter_context(tc.tile_pool(name="wstage", bufs=2))
        xload = es.enter_context(tc.tile_pool(name="xload", bufs=4))
        pswt = es.enter_context(tc.tile_pool(name="pswt", bufs=2, space="PSUM"))
        psx = es.enter_context(tc.tile_pool(name="psx", bufs=2, space="PSUM"))
        ps0p = es.enter_context(tc.tile_pool(name="ps0p", bufs=2, space="PSUM"))
        ps1p = es.enter_context(tc.tile_pool(name="ps1p", bufs=2, space="PSUM"))
        states = es.enter_context(tc.tile_pool(name="states", bufs=4))
        work = es.enter_context(tc.tile_pool(name="work", bufs=3))

        ident = consts.tile([P, P], F32)
        masks.make_identity(nc, ident[:])

        # ---- weight prep: transposed fp16 lhsT chunks ----
        whh0T = consts.tile([P, G4], F16)
        wih1T = consts.tile([P, G4], F16)
        whh1T = consts.tile([P, G4], F16)
        for src, dst in ((whh0, whh0T), (wih1, wih1T), (whh1, whh1T)):
            for g in range(4):
                wst = wstage.tile([P, H], F32, tag="wst")
                nc.sync.dma_start(out=wst[:], in_=src[g * P : (g + 1) * P, :])
                pst = pswt.tile([P, P], F32, tag="pswt")
                nc.tensor.transpose(pst[:], wst[:], ident[:])
                nc.scalar.copy(out=dst[:, g * P : (g + 1) * P], in_=pst[:])

        # wih0T augmented with the summed L0 bias as row 32 (compute-op start
        # partitions must be 32-aligned); rows 28..31 are zero so the
        # matching garbage rows of xT contribute nothing. K = KA = 33.
        wih0T = consts.tile([KA, G4], F16)
        nc.vector.memset(wih0T[:], 0.0)
        for g in range(4):
            wst = wstage.tile([P, D], F32, tag="wst")
            nc.sync.dma_start(out=wst[:], in_=wih0[g * P : (g + 1) * P, :])
            pst = psx.tile([D, P], F32, tag="psx")
            nc.tensor.transpose(pst[:], wst[:], ident[:])
            nc.vector.tensor_copy(wih0T[0:D, g * P : (g + 1) * P], pst[:])
        b0a = work.tile([1, G4], F32, tag="b0a")
        b0b = work.tile([1, G4], F32, tag="b0b")
        b0sum = work.tile([1, G4], F16, tag="b0sum")
        nc.sync.dma_start(out=b0a[:], in_=bih0)
        nc.sync.dma_start(out=b0b[:], in_=bhh0)
        nc.vector.tensor_add(b0sum[:], b0a[:], b0b[:])
        nc.sync.dma_start(out=wih0T[KA - 1 : KA, :], in_=b0sum[:])

        # L1 bias matmul operands: lhsT b4 [4, 128] (row r = summed bias of
        # weight chunk r), moving one-hot [4, 4B] with row r hot in column
        # block COL_OF[r].  psum += b4.T @ onehot puts bias b4[r, p] into
        # column block COL_OF[r] of partition p.
        b1a = work.tile([4, H], F32, tag="b1a")
        b1b = work.tile([4, H], F32, tag="b1b")
        nc.sync.dma_start(out=b1a[:], in_=bih1)
        nc.sync.dma_start(out=b1b[:], in_=bhh1)
        b4 = consts.tile([4, H], F16)
        nc.vector.tensor_add(b4[:], b1a[:], b1b[:])
        onehot = consts.tile([4, 4 * B], F16)
        nc.vector.memset(onehot[:], 0.0)
        ones_row = work.tile([1, B], F16, tag="ones_row")
        nc.vector.memset(ones_row[:], 1.0)
        for r in range(4):
            cb = COL_OF[r] * B
            nc.sync.dma_start(out=onehot[r : r + 1, cb : cb + B], in_=ones_row[:])

        # ---- x transpose prep, per quarter (xT stored fp16) ----
        xT = [
            consts.tile([KA, B * TQ], F16, tag=f"xT{q}", name=f"xT{q}")
            for q in range(Q)
        ]

        def emit_xprep(q, b):
            if b == 0:
                nc.vector.memset(xT[q][:], 0.0)
                nc.vector.memset(xT[q][KA - 1 : KA, :], 1.0)
            xt = xload.tile([TQ, D], F32, tag="xt")
            nc.sync.dma_start(out=xt[:], in_=x[b, q * TQ : (q + 1) * TQ, :])
            px = psx.tile([D, TQ], F32, tag="psx")
            nc.tensor.transpose(px[:], xt[:], ident[0:TQ, 0:TQ])
            dst = xT[q][0:D, b * TQ : (b + 1) * TQ]
            nc.vector.tensor_copy(dst, px[:])

        for b in range(B):
            emit_xprep(0, b)

        # ---- main time loop; L0 at t=k, L1 at t=k-2 ----
        # The 2-step lag lets every L1 matmul (bias, h1-recurrence on
        # h1(k-3), x-projection on h0(k-2)) run before h0(k-1) even exists,
        # so only L0's recurrence sits on the critical path.  L1's
        # elementwise runs on the otherwise-idle GpSimd engine; its tanh
        # stays on ACT but is queued after L0's critical ops.
        # h01 [128, 2B] fp16: cols 0:B = h0(k), B:2B = h1(k-2).
        # c01 [128, 2B] fp32: same split.
        h01 = states.tile([P, 2 * B], F16, tag="h01")
        c01 = states.tile([P, 2 * B], F16, tag="c01")
        nc.vector.memset(h01[:], 0.0)
        nc.vector.memset(c01[:], 0.0)
        h01_prev = h01  # becomes h01(k-2) view source below

        # interleave next-quarter x prep into the loop
        prep_schedule = {}  # iter k -> list of (q, b)
        if Q > 1:
            for q in range(1, Q):
                base = (q - 1) * TQ
                for b in range(B):
                    kk = base + (b * TQ) // B
                    prep_schedule.setdefault(kk, []).append((q, b))

        for k in range(t_steps + 2):
            h01_prev2, h01_prev, c01_prev = h01_prev, h01, c01
            h0_prev = h01_prev[:, 0:B]  # h0(k-1)
            h1_prev = h01_prev[:, B : 2 * B]  # h1(k-3)
            h0_prev2 = h01_prev2[:, 0:B]  # h0(k-2)

            do0 = k < t_steps
            do1 = k >= 2

            # ---------- PE: psum accumulation groups ----------
            ps0 = ps0p.tile([P, 4 * B], F32, tag="ps0", name="ps0") if do0 else None
            ps1 = ps1p.tile([P, 4 * B], F32, tag="ps1", name="ps1") if do1 else None

            if do1:
                # all of L1's matmuls depend only on >=2-iter-old state:
                # they pre-run while L0's chain finishes the previous step
                nc.tensor.matmul(
                    ps1[:], lhsT=b4[:], rhs=onehot[:], start=True, stop=False
                )
                for g in range(4):
                    cb = COL_OF[g] * B
                    nc.tensor.matmul(
                        ps1[:, cb : cb + B],
                        lhsT=whh1T[:, g * P : (g + 1) * P],
                        rhs=h1_prev,
                        start=False,
                        stop=False,
                    )
                for g in range(4):
                    cb = COL_OF[g] * B
                    nc.tensor.matmul(
                        ps1[:, cb : cb + B],
                        lhsT=wih1T[:, g * P : (g + 1) * P],
                        rhs=h0_prev2,
                        start=False,
                        stop=(g == 3),
                    )
            if do0:
                # early L0 work: x projection (+bias via ones row)
                q, tl = k // TQ, k % TQ
                rhs_x = xT[q][:].rearrange("p (b t) -> p t b", t=TQ)[:, tl, :]
                for g in range(4):
                    cb = COL_OF[g] * B
                    nc.tensor.matmul(
                        ps0[:, cb : cb + B],
                        lhsT=wih0T[:, g * P : (g + 1) * P],
                        rhs=rhs_x,
                        start=(g == 0),
                        stop=False,
                    )
                # critical path: L0 recurrent projection (needs h0_prev;
                # zeros at k=0, harmless)
                for g in range(4):
                    cb = COL_OF[g] * B
                    nc.tensor.matmul(
                        ps0[:, cb : cb + B],
                        lhsT=whh0T[:, g * P : (g + 1) * P],
                        rhs=h0_prev,
                        start=False,
                        stop=(g == 3),
                    )

            # ---------- elementwise; new state tiles ----------
            h01 = states.tile([P, 2 * B], F16, tag="h01")
            c01 = states.tile([P, 2 * B], F16, tag="c01")

            # --- L0 critical chain: ACT (split sigmoid) + DVE ---
            if do0:
                sifo0 = work.tile([P, 3 * B], F16, tag="sifo0")
                # i,f first (feed fc/ig); o last (only needed after tanh(c))
                nc.scalar.activation(sifo0[:, 0 : 2 * B], ps0[:, 0 : 2 * B], AF.Sigmoid)
                tg0 = work.tile([P, B], F16, tag="tg0")
                nc.scalar.activation(tg0[:], ps0[:, 3 * B : 4 * B], AF.Tanh)
                nc.scalar.activation(
                    sifo0[:, 2 * B : 3 * B], ps0[:, 2 * B : 3 * B], AF.Sigmoid
                )
                fc0 = work.tile([P, B], F16, tag="fc0")
                nc.vector.tensor_mul(fc0[:], sifo0[:, B : 2 * B], c01_prev[:, 0:B])
                ig0 = work.tile([P, B], F16, tag="ig0")
                nc.vector.tensor_mul(ig0[:], sifo0[:, 0:B], tg0[:])
                nc.vector.tensor_add(c01[:, 0:B], fc0[:], ig0[:])
                tc0 = work.tile([P, B], F16, tag="tc0")
                nc.scalar.activation(tc0[:], c01[:, 0:B], AF.Tanh)
                nc.vector.tensor_mul(h01[:, 0:B], sifo0[:, 2 * B : 3 * B], tc0[:])

            # --- L1 off-path: ACT sigmoids + GpSimd elementwise ---
            if do1:
                sifo1 = work.tile([P, 3 * B], F16, tag="sifo1")
                nc.scalar.activation(sifo1[:], ps1[:, 0 : 3 * B], AF.Sigmoid)
                tg1 = work.tile([P, B], F16, tag="tg1")
                nc.scalar.activation(tg1[:], ps1[:, 3 * B : 4 * B], AF.Tanh)
                fc1 = work.tile([P, B], F16, tag="fc1")
                nc.gpsimd.tensor_mul(fc1[:], sifo1[:, B : 2 * B], c01_prev[:, B : 2 * B])
                ig1 = work.tile([P, B], F16, tag="ig1")
                nc.gpsimd.tensor_mul(ig1[:], sifo1[:, 0:B], tg1[:])
                nc.gpsimd.tensor_add(c01[:, B : 2 * B], fc1[:], ig1[:])
                tc1 = work.tile([P, B], F16, tag="tc1")
                nc.scalar.activation(tc1[:], c01[:, B : 2 * B], AF.Tanh)
                nc.gpsimd.tensor_mul(h01[:, B : 2 * B], sifo1[:, 2 * B : 3 * B], tc1[:])
            else:
                nc.vector.memset(c01[:, B : 2 * B], 0.0)
                nc.vector.memset(h01[:, B : 2 * B], 0.0)

            # x prep for later quarters, queued after the chain ops
            for qb in prep_schedule.get(k, ()):
                emit_xprep(*qb)

        # ---- output: h1 = h01[:, B:2B] -> fp32 -> transpose -> [B, H] ----
        h1f = work.tile([P, B], F32, tag="h1f")
        nc.vector.tensor_copy(h1f[:], h01[:, B : 2 * B])
        pso = ps0p.tile([B, P], F32, tag="ps0")
        nc.tensor.transpose(pso[:], h1f[:], ident[:])
        ob = work.tile([B, P], F32, tag="ob")
        nc.vector.tensor_copy(ob[:], pso[:])
        nc.sync.dma_start(out=out, in_=ob[:])


_NC_CACHE = {}


def build_nc(t_steps=T_FULL):
    if t_steps in _NC_CACHE:
        return _NC_CACHE[t_steps]
    nc = bacc.Bacc(
        "TRN2",
        target_bir_lowering=False,
        debug=False,
        enable_asserts=False,
        num_devices=NCORES,
    )
    with tile.TileContext(nc) as tc:
        _emit(nc, tc, t_steps)
    nc.compile()
    _NC_CACHE[t_steps] = nc
    return nc


def make_in_maps(inputs, t_steps=T_FULL):
    x = np.asarray(inputs["x"], dtype=np.float32).reshape(B_FULL, T_FULL, D)
    x = x[:, :t_steps, :]
    shared = {
        "W_ih0": np.ascontiguousarray(inputs["W_ih0"], dtype=np.float32),
        "W_hh0": np.ascontiguousarray(inputs["W_hh0"], dtype=np.float32),
        "b_ih0": np.asarray(inputs["b_ih0"], np.float32).reshape(1, G4),
        "b_hh0": np.asarray(inputs["b_hh0"], np.float32).reshape(1, G4),
        "W_ih1": np.ascontiguousarray(inputs["W_ih1"], dtype=np.float32),
        "W_hh1": np.ascontiguousarray(inputs["W_hh1"], dtype=np.float32),
        "b_ih1": np.asarray(inputs["b_ih1"], np.float32).reshape(4, H),
        "b_hh1": np.asarray(inputs["b_hh1"], np.float32).reshape(4, H),
    }
    in_maps = []
    for c in range(NCORES):
        m = dict(shared)
        m["x"] = np.ascontiguousarray(x[c * B : (c + 1) * B])
        in_maps.append(m)
    return in_maps


def run(inputs, t_steps=T_FULL, trace=False, **kwargs):
    nc = build_nc(t_steps)
    in_maps = make_in_maps(inputs, t_steps)
    res = run_bass_kernel_spmd(
        nc, in_maps, core_ids=list(range(NCORES)), trace=trace, **kwargs
    )
    outs = [res.results[c]["out"] for c in range(NCORES)]
    return np.concatenate(outs, axis=0).astype(np.float32), res


def kernel(**inputs):
    out, _ = run(inputs)
    return out
